# revision 20
# baseline (speedup 1.0000x reference)
"""Trainium2 Bass kernel for ContextQueryAttention (trilinear similarity +
row/col softmax attention).

Full-input contract: kernel(**inputs) takes the complete arrays
  q  [16, 128, 512]   f32
  c  [16, 128, 4096]  f32
  w1 [1, 128] w2 [1, 128] w3 [1, 128] f32
and returns out [16, 512, 4096] f32 = concat([c, a, c*a, c*b], axis=1).

Sharding: data-parallel over batch B=16 across 8 NeuronCores (2 batches per
core), no collectives.

Math notes:
  s[n,m] = out3[n,m] + out1[m] + out2[n]
  [m,n] pass (true exp): sT = (w3*q+w2)^T @ c, bias out1[m] per-partition;
    colsum accumulates for free in the ACT instruction.
  [n,m] pass computes exp(s - out2[n]) = exp((w3*c+w1)^T @ q) with NO bias
    (out2 is constant along the row-softmax axis so it cancels in sRow);
    the missing e^{out2[n]} factor is folded into the transposed-c
    stationary of the tmp matmul: cE[n,d] = c[d,n]^T * e^{out2[n]}, applied
    as a stride-0-broadcast tensor_tensor per 4-chunk transpose quad.
  rowsum' (column layout [nn, j]) via DVE reduce over expS'; reciprocal is
    only cheap at [128,32] (HW reciprocal ~5.6 cyc/element, free-size
    scaled); rowinv = 1/(rowsum'*e2) -> PE transpose -> [1,N] row -> gpsimd
    partition_broadcast -> rowinvb [128,N].
  cE/tmp are emitted before the rowsum reduce so their PE work covers the
    17us DVE reduce; pass 1's ACT-paced phase covers the rowinv broadcast
    chain. Loads for both batches are issued up front.
  All matmul operands are bf16 (1 cyc/row, cheap LDWEIGHTS); exp outputs are
  written bf16 directly by ACT (free cast).
"""

import sys

import numpy as np

try:
    import concourse.bass as bass  # noqa: F401
except Exception:  # pragma: no cover
    sys.path.insert(0, "/opt/trn_rl_repo")
    import concourse.bass as bass  # noqa: F401

import concourse.bacc as bacc
import concourse.mybir as mybir
import concourse.tile as tile
from concourse.masks import make_identity

F32 = mybir.dt.float32
BF16 = mybir.dt.bfloat16

# Problem geometry (hardcoded per contract)
B = 16          # total batches
NCORES = 8
CB = B // NCORES  # batches per core = 2
D = 128         # model dim == partition count
M = 512         # query length
N = 4096        # context length
P = 128
NCH = N // P    # 32 n-chunks of 128
MCH = M // P    # 4 m-chunks of 128
NW = N // 1024  # 4 wide column groups of 1024


def build_body(tc, q_ap, c_ap, w1_ap, w2_ap, w3_ap, out_ap):
    """Emit the per-core program. q_ap [CB,128,512], c_ap [CB,128,4096],
    w*_ap [1,128], out_ap [CB,512,4096]."""
    from contextlib import ExitStack

    nc = tc.nc
    mult = mybir.AluOpType.mult
    add = mybir.AluOpType.add
    Exp = mybir.ActivationFunctionType.Exp
    AxX = mybir.AxisListType.X

    with ExitStack() as ctx:
        consts = ctx.enter_context(tc.tile_pool(name="consts", bufs=1))
        cq = ctx.enter_context(tc.tile_pool(name="cq", bufs=2))
        small = ctx.enter_context(tc.tile_pool(name="small", bufs=2))
        big = ctx.enter_context(tc.tile_pool(name="big", bufs=1))
        outp = ctx.enter_context(tc.tile_pool(name="outp", bufs=2))
        pp_s = ctx.enter_context(tc.tile_pool(name="pp_s", bufs=2, space="PSUM"))
        pp_tr = ctx.enter_context(tc.tile_pool(name="pp_tr", bufs=2, space="PSUM"))
        pp_tmp = ctx.enter_context(tc.tile_pool(name="pp_tmp", bufs=1, space="PSUM"))

        identity = consts.tile([P, P], F32)
        make_identity(nc, identity)
        identity_b = consts.tile([P, P], BF16)
        nc.vector.tensor_copy(identity_b, identity)
        w1c = consts.tile([P, 1], F32)
        w2c = consts.tile([P, 1], F32)
        w3c = consts.tile([P, 1], F32)
        nc.sync.dma_start(out=w1c, in_=w1_ap.rearrange("o d -> d o"))
        nc.sync.dma_start(out=w2c, in_=w2_ap.rearrange("o d -> d o"))
        nc.sync.dma_start(out=w3c, in_=w3_ap.rearrange("o d -> d o"))
        w1b = consts.tile([P, 1], BF16)
        w2b = consts.tile([P, 1], BF16)
        nc.vector.tensor_copy(w1b, w1c)
        nc.vector.tensor_copy(w2b, w2c)

        # ---- loads for BOTH batches up front (cq bufs=2): keeps batch 1's
        # cast-DMAs from queueing behind batch 0's gpsimd compute ----
        q_bs, c_bs, c_ts = [], [], []
        for b in range(CB):
            q_b = small.tile([P, M], BF16, tag="qb", name=f"qb{b}")
            nc.gpsimd.dma_start(out=q_b, in_=q_ap[b])
            c_b = cq.tile([P, N], BF16, tag="cb", name=f"cb{b}")
            for w in range(NW):
                nc.gpsimd.dma_start(
                    out=c_b[:, w * 1024 : (w + 1) * 1024],
                    in_=c_ap[b][:, w * 1024 : (w + 1) * 1024],
                )
            c_t = cq.tile([P, N], F32, tag="c", name=f"ct{b}")
            nc.sync.dma_start(out=c_t, in_=c_ap[b])
            q_bs.append(q_b)
            c_bs.append(c_b)
            c_ts.append(c_t)

        for b in range(CB):
            q_b, c_b, c_t = q_bs[b], c_bs[b], c_ts[b]

            # output block 0 is just c (issue early; 4 chunks)
            for w in range(NW):
                nc.sync.dma_start(
                    out=out_ap[b, 0:P, w * 1024 : (w + 1) * 1024],
                    in_=c_t[:, w * 1024 : (w + 1) * 1024],
                )

            # ---- folded lhsT tensors ----
            Bq_b = small.tile([P, M], BF16, tag="Bq")
            nc.vector.tensor_scalar(Bq_b, q_b, w3c, w2c, mult, add)
            A_b = big.tile([P, N], BF16, tag="A", name=f"A{b}")
            for w in range(NW):
                nc.vector.tensor_scalar(
                    A_b[:, w * 1024 : (w + 1) * 1024],
                    c_b[:, w * 1024 : (w + 1) * 1024],
                    w3c,
                    w1c,
                    mult,
                    add,
                )

            # ---- qT (PE transpose, bf16) ----
            qT_b = small.tile([P, M], BF16, tag="qT")
            for i in range(MCH):
                ps_q = pp_tr.tile([P, P], BF16, tag="tr")
                nc.tensor.transpose(ps_q, q_b[:, i * P : (i + 1) * P], identity_b)
                nc.vector.tensor_copy(qT_b[:, i * P : (i + 1) * P], ps_q)

            # ---- out1col[m] ----
            ps_o1 = pp_tr.tile([P, MCH], F32, tag="tr")
            for i in range(MCH):
                nc.tensor.matmul(
                    ps_o1[:, i : i + 1],
                    lhsT=q_b[:, i * P : (i + 1) * P],
                    rhs=w1b,
                    start=True,
                    stop=True,
                )
            out1col = small.tile([P, MCH], F32, tag="o1")
            nc.vector.tensor_copy(out1col, ps_o1)

            # ---- out2col[n] -> e2 ----
            ps_o2 = pp_tr.tile([P, NCH], F32, tag="tr")
            for j in range(NCH):
                nc.tensor.matmul(
                    ps_o2[:, j : j + 1],
                    lhsT=c_b[:, j * P : (j + 1) * P],
                    rhs=w2b,
                    start=True,
                    stop=True,
                )
            e2col = small.tile([P, NCH], F32, tag="e2")
            nc.scalar.activation(e2col, ps_o2, Exp, bias=0.0, scale=1.0)

            # ---- pass 2: [n,m] layout, exp(s - out2[n]) (no bias) ----
            expS_b = big.tile([P, NCH, M], BF16, tag="expS")
            for jj in range(NCH // 2):
                ps2 = pp_s.tile([P, 1024], F32, tag="s")
                for h in range(2):
                    j = 2 * jj + h
                    nc.tensor.matmul(
                        ps2[:, h * M : (h + 1) * M],
                        lhsT=A_b[:, j * P : (j + 1) * P],
                        rhs=q_b,
                        start=True,
                        stop=True,
                    )
                nc.scalar.activation(
                    expS_b[:, 2 * jj : 2 * jj + 2, :], ps2, Exp, bias=0.0, scale=1.0
                )

            # ---- cE quads + tmp matmuls (PE work covers the DVE reduce) ----
            cE_b = big.tile([P, NCH, P], BF16, tag="cE", name=f"cE{b}")
            ps_tmp = pp_tmp.tile([P, M], F32, tag="tmp")

            def quad(x):
                ps_ct = pp_tr.tile([P, 4, P], BF16, tag="tr4", bufs=1)
                for k in range(4):
                    j = 4 * x + k
                    nc.tensor.transpose(
                        ps_ct[:, k, :], c_b[:, j * P : (j + 1) * P], identity_b
                    )
                nc.vector.tensor_tensor(
                    cE_b[:, 4 * x : 4 * x + 4, :],
                    ps_ct,
                    e2col[:, 4 * x : 4 * x + 4].broadcast_to([P, 4, P]),
                    mult,
                )

            quad(0)
            for x in range(NCH // 4):
                if x + 1 < NCH // 4:
                    quad(x + 1)
                for k in range(4):
                    j = 4 * x + k
                    nc.tensor.matmul(
                        ps_tmp,
                        lhsT=cE_b[:, j, :],
                        rhs=expS_b[:, j, :],
                        start=(j == 0),
                        stop=(j == NCH - 1),
                    )

            # ---- rowsum' reduce + rowinv chain ----
            rowsumC = small.tile([P, NCH], F32, tag="rsC")
            HN = NCH // 2
            nc.vector.reduce_sum(rowsumC[:, 0:HN], expS_b[:, 0:HN, :], axis=AxX)
            nc.vector.reduce_sum(rowsumC[:, HN:NCH], expS_b[:, HN:NCH, :], axis=AxX)
            rowprod = small.tile([P, NCH], F32, tag="rp")
            nc.vector.tensor_tensor(rowprod, rowsumC, e2col, mult)
            rowinvC = small.tile([P, NCH], F32, tag="ri")
            nc.vector.reciprocal(rowinvC, rowprod)
            ps_rT = pp_tr.tile([NCH, P], F32, tag="tr")
            nc.tensor.transpose(ps_rT, rowinvC, identity)
            rowT = small.tile([NCH, P], F32, tag="rT")
            nc.vector.tensor_copy(rowT, ps_rT)
            rowrow = big.tile([1, N], F32, tag="rowrow")
            nc.sync.dma_start(
                out=rowrow.rearrange("p (a b) -> p a b", a=NCH), in_=rowT
            )
            rowinvb = big.tile([P, N], F32, tag="rowinvb")
            nc.gpsimd.partition_broadcast(rowinvb, rowrow)

            # ---- pass 1: [m,n] layout, TRUE exp with out1 bias + colsum ----
            expST_b = big.tile([P, MCH, N], BF16, tag="expST")
            colsumU = small.tile([P, MCH, NW], F32, tag="csU")
            for jw in range(NW):
                for i in range(MCH):
                    ps1 = pp_s.tile([P, 1024], F32, tag="s")
                    for h in range(2):
                        nc.tensor.matmul(
                            ps1[:, h * M : (h + 1) * M],
                            lhsT=Bq_b[:, i * P : (i + 1) * P],
                            rhs=c_b[:, jw * 1024 + h * M : jw * 1024 + (h + 1) * M],
                            start=True,
                            stop=True,
                        )
                    nc.scalar.activation(
                        expST_b[:, i, jw * 1024 : (jw + 1) * 1024],
                        ps1,
                        Exp,
                        bias=out1col[:, i : i + 1],
                        scale=1.0,
                        accum_out=colsumU[:, i, jw : jw + 1],
                    )

            # ---- a output waves ----
            for w in range(NW):
                lo = w * 1024
                ps_a = pp_s.tile([P, 1024], F32, tag="s")
                for i in range(MCH):
                    for h in range(2):
                        nc.tensor.matmul(
                            ps_a[:, h * M : (h + 1) * M],
                            lhsT=qT_b[:, i * P : (i + 1) * P],
                            rhs=expST_b[:, i, lo + h * M : lo + (h + 1) * M],
                            start=(i == 0),
                            stop=(i == MCH - 1),
                        )
                a_t = outp.tile([P, 1024], F32, tag="a")
                nc.vector.tensor_tensor(a_t, ps_a, rowinvb[:, lo : lo + 1024], mult)
                nc.sync.dma_start(out=out_ap[b, P : 2 * P, lo : lo + 1024], in_=a_t)
                ca_t = outp.tile([P, 1024], F32, tag="ca")
                nc.gpsimd.tensor_tensor(ca_t, a_t, c_t[:, lo : lo + 1024], mult)
                nc.gpsimd.dma_start(
                    out=out_ap[b, 2 * P : 3 * P, lo : lo + 1024], in_=ca_t
                )

            # ---- column softmax stats + tmpT ----
            colsum = small.tile([P, MCH], F32, tag="cs")
            nc.vector.reduce_sum(colsum, colsumU, axis=AxX)
            colinv = small.tile([P, MCH], F32, tag="ci")
            nc.vector.reciprocal(colinv, colsum)
            tmpUb = small.tile([P, M], BF16, tag="tmpU")
            nc.vector.tensor_copy(tmpUb, ps_tmp)
            tmpT_b = small.tile([P, M], BF16, tag="tmpT")
            for i in range(MCH):
                ps_tt = pp_tr.tile([P, P], BF16, tag="tr")
                nc.tensor.transpose(ps_tt, tmpUb[:, i * P : (i + 1) * P], identity_b)
                nc.vector.tensor_scalar(
                    tmpT_b[:, i * P : (i + 1) * P],
                    ps_tt,
                    colinv[:, i : i + 1],
                    None,
                    mult,
                )

            # ---- b output waves ----
            for w in range(NW):
                lo = w * 1024
                ps_b = pp_s.tile([P, 1024], F32, tag="s")
                for i in range(MCH):
                    for h in range(2):
                        nc.tensor.matmul(
                            ps_b[:, h * M : (h + 1) * M],
                            lhsT=tmpT_b[:, i * P : (i + 1) * P],
                            rhs=expST_b[:, i, lo + h * M : lo + (h + 1) * M],
                            start=(i == 0),
                            stop=(i == MCH - 1),
                        )
                b1_t = outp.tile([P, 1024], F32, tag="b1")
                nc.vector.tensor_tensor(b1_t, ps_b, rowinvb[:, lo : lo + 1024], mult)
                cb_t = outp.tile([P, 1024], F32, tag="cb")
                nc.gpsimd.tensor_tensor(cb_t, b1_t, c_t[:, lo : lo + 1024], mult)
                nc.gpsimd.dma_start(
                    out=out_ap[b, 3 * P : 4 * P, lo : lo + 1024], in_=cb_t
                )


_PROGRAM = None


def _build_program(loops=None):
    """Build the per-core Bass program. loops=None -> straight-line (grading
    path); loops=R -> wrap the body in a Tile For_i repetition loop (used
    only for steady-state benchmarking)."""
    nc = bacc.Bacc("TRN2", target_bir_lowering=False, debug=False, num_devices=NCORES)
    q_d = nc.dram_tensor("q", [CB, D, M], F32, kind="ExternalInput")
    c_d = nc.dram_tensor("c", [CB, D, N], F32, kind="ExternalInput")
    w1_d = nc.dram_tensor("w1", [1, D], F32, kind="ExternalInput")
    w2_d = nc.dram_tensor("w2", [1, D], F32, kind="ExternalInput")
    w3_d = nc.dram_tensor("w3", [1, D], F32, kind="ExternalInput")
    out_d = nc.dram_tensor("out", [CB, 4 * D, N], F32, kind="ExternalOutput")
    with tile.TileContext(nc) as tc:
        if loops is None:
            build_body(
                tc, q_d.ap(), c_d.ap(), w1_d.ap(), w2_d.ap(), w3_d.ap(), out_d.ap()
            )
        else:
            with tc.For_i(0, loops, 1):
                build_body(
                    tc,
                    q_d.ap(),
                    c_d.ap(),
                    w1_d.ap(),
                    w2_d.ap(),
                    w3_d.ap(),
                    out_d.ap(),
                )
    nc.compile()
    return nc


def _get_program():
    global _PROGRAM
    if _PROGRAM is None:
        _PROGRAM = _build_program()
    return _PROGRAM


def kernel(q, c, w1, w2, w3, _collect_results=None):
    q = np.ascontiguousarray(q, dtype=np.float32)
    c = np.ascontiguousarray(c, dtype=np.float32)
    w1 = np.ascontiguousarray(w1, dtype=np.float32)
    w2 = np.ascontiguousarray(w2, dtype=np.float32)
    w3 = np.ascontiguousarray(w3, dtype=np.float32)

    nc = _get_program()
    in_maps = [
        {
            "q": q[CB * i : CB * (i + 1)],
            "c": c[CB * i : CB * (i + 1)],
            "w1": w1,
            "w2": w2,
            "w3": w3,
        }
        for i in range(NCORES)
    ]
    from concourse import bass_utils

    res = bass_utils.run_bass_kernel_spmd(nc, in_maps, core_ids=list(range(NCORES)))
    if _collect_results is not None:
        _collect_results.append(res)
    return np.concatenate([r["out"] for r in res.results], axis=0)


# revision 22
# speedup vs baseline: 1.1578x; 1.1578x over previous
"""Trainium2 Bass kernel for ContextQueryAttention (trilinear similarity +
row/col softmax attention).

Full-input contract: kernel(**inputs) takes the complete arrays
  q  [16, 128, 512]   f32
  c  [16, 128, 4096]  f32
  w1 [1, 128] w2 [1, 128] w3 [1, 128] f32
and returns out [16, 512, 4096] f32 = concat([c, a, c*a, c*b], axis=1).

Sharding: data-parallel over batch B=16 across 8 NeuronCores (2 batches per
core), no collectives.

Math notes:
  s[n,m] = out3[n,m] + out1[m] + out2[n]
  [m,n] pass (true exp): sT = (w3*q+w2)^T @ c, bias out1[m] per-partition;
    colsum accumulates for free in the ACT instruction.
  [n,m] pass computes exp(s - out2[n]) = exp((w3*c+w1)^T @ q) with NO bias
    (out2 is constant along the row-softmax axis so it cancels in sRow);
    the missing e^{out2[n]} factor is folded into the transposed-c
    stationary of the tmp matmul: cE[n,d] = c[d,n]^T * e^{out2[n]}, applied
    as a stride-0-broadcast tensor_tensor per 4-chunk transpose quad.
  rowsum' (column layout [nn, j]) via DVE reduce over expS'; reciprocal is
    only cheap at [128,32] (HW reciprocal ~5.6 cyc/element, free-size
    scaled); rowinv = 1/(rowsum'*e2) -> PE transpose -> [1,N] row -> gpsimd
    partition_broadcast -> rowinvb [128,N].
  cE/tmp are emitted before the rowsum reduce so their PE work covers the
    17us DVE reduce; pass 1's ACT-paced phase covers the rowinv broadcast
    chain. Loads for both batches are issued up front.
  All matmul operands are bf16 (1 cyc/row, cheap LDWEIGHTS); exp outputs are
  written bf16 directly by ACT (free cast).
"""

import sys

import numpy as np

try:
    import concourse.bass as bass  # noqa: F401
except Exception:  # pragma: no cover
    sys.path.insert(0, "/opt/trn_rl_repo")
    import concourse.bass as bass  # noqa: F401

import concourse.bacc as bacc
import concourse.mybir as mybir
import concourse.tile as tile
from concourse.masks import make_identity

F32 = mybir.dt.float32
BF16 = mybir.dt.bfloat16

# Problem geometry (hardcoded per contract)
B = 16          # total batches
NCORES = 8
CB = B // NCORES  # batches per core = 2
D = 128         # model dim == partition count
M = 512         # query length
N = 4096        # context length
P = 128
NCH = N // P    # 32 n-chunks of 128
MCH = M // P    # 4 m-chunks of 128
NW = N // 1024  # 4 wide column groups of 1024


def build_body(tc, q_ap, c_ap, w1_ap, w2_ap, w3_ap, out_ap):
    """Emit the per-core program. q_ap [CB,128,512], c_ap [CB,128,4096],
    w*_ap [1,128], out_ap [CB,512,4096]."""
    from contextlib import ExitStack

    nc = tc.nc
    mult = mybir.AluOpType.mult
    add = mybir.AluOpType.add
    Exp = mybir.ActivationFunctionType.Exp
    AxX = mybir.AxisListType.X

    with ExitStack() as ctx:
        consts = ctx.enter_context(tc.tile_pool(name="consts", bufs=1))
        cq = ctx.enter_context(tc.tile_pool(name="cq", bufs=2))
        small = ctx.enter_context(tc.tile_pool(name="small", bufs=2))
        big = ctx.enter_context(tc.tile_pool(name="big", bufs=1))
        outp = ctx.enter_context(tc.tile_pool(name="outp", bufs=2))
        pp_s = ctx.enter_context(tc.tile_pool(name="pp_s", bufs=2, space="PSUM"))
        pp_tr = ctx.enter_context(tc.tile_pool(name="pp_tr", bufs=2, space="PSUM"))
        pp_tmp = ctx.enter_context(tc.tile_pool(name="pp_tmp", bufs=1, space="PSUM"))

        identity = consts.tile([P, P], F32)
        make_identity(nc, identity)
        identity_b = consts.tile([P, P], BF16)
        nc.vector.tensor_copy(identity_b, identity)
        w1c = consts.tile([P, 1], F32)
        w2c = consts.tile([P, 1], F32)
        w3c = consts.tile([P, 1], F32)
        nc.sync.dma_start(out=w1c, in_=w1_ap.rearrange("o d -> d o"))
        nc.sync.dma_start(out=w2c, in_=w2_ap.rearrange("o d -> d o"))
        nc.sync.dma_start(out=w3c, in_=w3_ap.rearrange("o d -> d o"))
        w1b = consts.tile([P, 1], BF16)
        w2b = consts.tile([P, 1], BF16)
        nc.vector.tensor_copy(w1b, w1c)
        nc.vector.tensor_copy(w2b, w2c)

        for b in range(CB):
            # ---- loads ----
            q_b = small.tile([P, M], BF16, tag="qb", name=f"qb{b}")
            nc.gpsimd.dma_start(out=q_b, in_=q_ap[b])
            c_b = cq.tile([P, N], BF16, tag="cb", name=f"cb{b}")
            for w in range(NW):
                nc.gpsimd.dma_start(
                    out=c_b[:, w * 1024 : (w + 1) * 1024],
                    in_=c_ap[b][:, w * 1024 : (w + 1) * 1024],
                )
            c_t = cq.tile([P, N], F32, tag="c", name=f"ct{b}")
            nc.sync.dma_start(out=c_t, in_=c_ap[b])

            # output block 0 is just c (issue early; 4 chunks)
            for w in range(NW):
                nc.sync.dma_start(
                    out=out_ap[b, 0:P, w * 1024 : (w + 1) * 1024],
                    in_=c_t[:, w * 1024 : (w + 1) * 1024],
                )

            # ---- folded lhsT tensors ----
            Bq_b = small.tile([P, M], BF16, tag="Bq")
            nc.vector.tensor_scalar(Bq_b, q_b, w3c, w2c, mult, add)
            A_b = big.tile([P, N], BF16, tag="A", name=f"A{b}")
            for w in range(NW):
                nc.vector.tensor_scalar(
                    A_b[:, w * 1024 : (w + 1) * 1024],
                    c_b[:, w * 1024 : (w + 1) * 1024],
                    w3c,
                    w1c,
                    mult,
                    add,
                )

            # ---- qT (PE transpose, bf16) ----
            qT_b = small.tile([P, M], BF16, tag="qT")
            for i in range(MCH):
                ps_q = pp_tr.tile([P, P], BF16, tag="tr")
                nc.tensor.transpose(ps_q, q_b[:, i * P : (i + 1) * P], identity_b)
                nc.vector.tensor_copy(qT_b[:, i * P : (i + 1) * P], ps_q)

            # ---- out1col[m] ----
            ps_o1 = pp_tr.tile([P, MCH], F32, tag="tr")
            for i in range(MCH):
                nc.tensor.matmul(
                    ps_o1[:, i : i + 1],
                    lhsT=q_b[:, i * P : (i + 1) * P],
                    rhs=w1b,
                    start=True,
                    stop=True,
                )
            out1col = small.tile([P, MCH], F32, tag="o1")
            nc.vector.tensor_copy(out1col, ps_o1)

            # ---- out2col[n] -> e2 ----
            ps_o2 = pp_tr.tile([P, NCH], F32, tag="tr")
            for j in range(NCH):
                nc.tensor.matmul(
                    ps_o2[:, j : j + 1],
                    lhsT=c_b[:, j * P : (j + 1) * P],
                    rhs=w2b,
                    start=True,
                    stop=True,
                )
            e2col = small.tile([P, NCH], F32, tag="e2")
            nc.scalar.activation(e2col, ps_o2, Exp, bias=0.0, scale=1.0)

            # ---- pass 2: [n,m] layout, exp(s - out2[n]) (no bias) ----
            expS_b = big.tile([P, NCH, M], BF16, tag="expS")
            for jj in range(NCH // 2):
                ps2 = pp_s.tile([P, 1024], F32, tag="s")
                for h in range(2):
                    j = 2 * jj + h
                    nc.tensor.matmul(
                        ps2[:, h * M : (h + 1) * M],
                        lhsT=A_b[:, j * P : (j + 1) * P],
                        rhs=q_b,
                        start=True,
                        stop=True,
                    )
                nc.scalar.activation(
                    expS_b[:, 2 * jj : 2 * jj + 2, :], ps2, Exp, bias=0.0, scale=1.0
                )

            # ---- rowsum' reduce + rowinv chain ----
            rowsumC = small.tile([P, NCH], F32, tag="rsC")
            HN = NCH // 2
            nc.vector.reduce_sum(rowsumC[:, 0:HN], expS_b[:, 0:HN, :], axis=AxX)
            nc.vector.reduce_sum(rowsumC[:, HN:NCH], expS_b[:, HN:NCH, :], axis=AxX)
            rowprod = small.tile([P, NCH], F32, tag="rp")
            nc.vector.tensor_tensor(rowprod, rowsumC, e2col, mult)
            rowinvC = small.tile([P, NCH], F32, tag="ri")
            nc.vector.reciprocal(rowinvC, rowprod)
            ps_rT = pp_tr.tile([NCH, P], F32, tag="tr")
            nc.tensor.transpose(ps_rT, rowinvC, identity)
            rowT = small.tile([NCH, P], F32, tag="rT")
            nc.vector.tensor_copy(rowT, ps_rT)
            rowrow = big.tile([1, N], F32, tag="rowrow")
            nc.sync.dma_start(
                out=rowrow.rearrange("p (a b) -> p a b", a=NCH), in_=rowT
            )
            rowinvb = big.tile([P, N], F32, tag="rowinvb")
            nc.gpsimd.partition_broadcast(rowinvb, rowrow)

            # ---- cE[n,d] = c^T * e2[n] ----
            cE_b = big.tile([P, NCH, P], BF16, tag="cE", name=f"cE{b}")
            for x in range(NCH // 4):
                ps_ct = pp_tr.tile([P, 4, P], BF16, tag="tr4", bufs=1)
                for k in range(4):
                    j = 4 * x + k
                    nc.tensor.transpose(
                        ps_ct[:, k, :], c_b[:, j * P : (j + 1) * P], identity_b
                    )
                nc.vector.tensor_tensor(
                    cE_b[:, 4 * x : 4 * x + 4, :],
                    ps_ct,
                    e2col[:, 4 * x : 4 * x + 4].broadcast_to([P, 4, P]),
                    mult,
                )

            # ---- tmp[d,m] = sum_n cE[n,d] expS'[n,m] ----
            ps_tmp = pp_tmp.tile([P, M], F32, tag="tmp")
            for j in range(NCH):
                nc.tensor.matmul(
                    ps_tmp,
                    lhsT=cE_b[:, j, :],
                    rhs=expS_b[:, j, :],
                    start=(j == 0),
                    stop=(j == NCH - 1),
                )

            # ---- pass 1: [m,n] layout, TRUE exp with out1 bias + colsum ----
            expST_b = big.tile([P, MCH, N], BF16, tag="expST")
            colsumU = small.tile([P, MCH, NW], F32, tag="csU")
            for jw in range(NW):
                for i in range(MCH):
                    ps1 = pp_s.tile([P, 1024], F32, tag="s")
                    for h in range(2):
                        nc.tensor.matmul(
                            ps1[:, h * M : (h + 1) * M],
                            lhsT=Bq_b[:, i * P : (i + 1) * P],
                            rhs=c_b[:, jw * 1024 + h * M : jw * 1024 + (h + 1) * M],
                            start=True,
                            stop=True,
                        )
                    nc.scalar.activation(
                        expST_b[:, i, jw * 1024 : (jw + 1) * 1024],
                        ps1,
                        Exp,
                        bias=out1col[:, i : i + 1],
                        scale=1.0,
                        accum_out=colsumU[:, i, jw : jw + 1],
                    )

            # ---- a output waves ----
            for w in range(NW):
                lo = w * 1024
                ps_a = pp_s.tile([P, 1024], F32, tag="s")
                for i in range(MCH):
                    for h in range(2):
                        nc.tensor.matmul(
                            ps_a[:, h * M : (h + 1) * M],
                            lhsT=qT_b[:, i * P : (i + 1) * P],
                            rhs=expST_b[:, i, lo + h * M : lo + (h + 1) * M],
                            start=(i == 0),
                            stop=(i == MCH - 1),
                        )
                a_t = outp.tile([P, 1024], F32, tag="a")
                nc.vector.tensor_tensor(a_t, ps_a, rowinvb[:, lo : lo + 1024], mult)
                nc.sync.dma_start(out=out_ap[b, P : 2 * P, lo : lo + 1024], in_=a_t)
                ca_t = outp.tile([P, 1024], F32, tag="ca")
                nc.gpsimd.tensor_tensor(ca_t, a_t, c_t[:, lo : lo + 1024], mult)
                nc.gpsimd.dma_start(
                    out=out_ap[b, 2 * P : 3 * P, lo : lo + 1024], in_=ca_t
                )

            # ---- column softmax stats + tmpT ----
            colsum = small.tile([P, MCH], F32, tag="cs")
            nc.vector.reduce_sum(colsum, colsumU, axis=AxX)
            colinv = small.tile([P, MCH], F32, tag="ci")
            nc.vector.reciprocal(colinv, colsum)
            tmpUb = small.tile([P, M], BF16, tag="tmpU")
            nc.vector.tensor_copy(tmpUb, ps_tmp)
            tmpT_b = small.tile([P, M], BF16, tag="tmpT")
            for i in range(MCH):
                ps_tt = pp_tr.tile([P, P], BF16, tag="tr")
                nc.tensor.transpose(ps_tt, tmpUb[:, i * P : (i + 1) * P], identity_b)
                nc.vector.tensor_scalar(
                    tmpT_b[:, i * P : (i + 1) * P],
                    ps_tt,
                    colinv[:, i : i + 1],
                    None,
                    mult,
                )

            # ---- b output waves ----
            for w in range(NW):
                lo = w * 1024
                ps_b = pp_s.tile([P, 1024], F32, tag="s")
                for i in range(MCH):
                    for h in range(2):
                        nc.tensor.matmul(
                            ps_b[:, h * M : (h + 1) * M],
                            lhsT=tmpT_b[:, i * P : (i + 1) * P],
                            rhs=expST_b[:, i, lo + h * M : lo + (h + 1) * M],
                            start=(i == 0),
                            stop=(i == MCH - 1),
                        )
                b1_t = outp.tile([P, 1024], F32, tag="b1")
                nc.vector.tensor_tensor(b1_t, ps_b, rowinvb[:, lo : lo + 1024], mult)
                cb_t = outp.tile([P, 1024], F32, tag="cb")
                nc.gpsimd.tensor_tensor(cb_t, b1_t, c_t[:, lo : lo + 1024], mult)
                nc.gpsimd.dma_start(
                    out=out_ap[b, 3 * P : 4 * P, lo : lo + 1024], in_=cb_t
                )


_PROGRAM = None


def _build_program(loops=None):
    """Build the per-core Bass program. loops=None -> straight-line (grading
    path); loops=R -> wrap the body in a Tile For_i repetition loop (used
    only for steady-state benchmarking)."""
    nc = bacc.Bacc("TRN2", target_bir_lowering=False, debug=False, num_devices=NCORES)
    q_d = nc.dram_tensor("q", [CB, D, M], F32, kind="ExternalInput")
    c_d = nc.dram_tensor("c", [CB, D, N], F32, kind="ExternalInput")
    w1_d = nc.dram_tensor("w1", [1, D], F32, kind="ExternalInput")
    w2_d = nc.dram_tensor("w2", [1, D], F32, kind="ExternalInput")
    w3_d = nc.dram_tensor("w3", [1, D], F32, kind="ExternalInput")
    out_d = nc.dram_tensor("out", [CB, 4 * D, N], F32, kind="ExternalOutput")
    with tile.TileContext(nc) as tc:
        if loops is None:
            build_body(
                tc, q_d.ap(), c_d.ap(), w1_d.ap(), w2_d.ap(), w3_d.ap(), out_d.ap()
            )
        else:
            with tc.For_i(0, loops, 1):
                build_body(
                    tc,
                    q_d.ap(),
                    c_d.ap(),
                    w1_d.ap(),
                    w2_d.ap(),
                    w3_d.ap(),
                    out_d.ap(),
                )
    nc.compile()
    return nc


def _get_program():
    global _PROGRAM
    if _PROGRAM is None:
        _PROGRAM = _build_program()
    return _PROGRAM


def kernel(q, c, w1, w2, w3, _collect_results=None):
    q = np.ascontiguousarray(q, dtype=np.float32)
    c = np.ascontiguousarray(c, dtype=np.float32)
    w1 = np.ascontiguousarray(w1, dtype=np.float32)
    w2 = np.ascontiguousarray(w2, dtype=np.float32)
    w3 = np.ascontiguousarray(w3, dtype=np.float32)

    nc = _get_program()
    in_maps = [
        {
            "q": q[CB * i : CB * (i + 1)],
            "c": c[CB * i : CB * (i + 1)],
            "w1": w1,
            "w2": w2,
            "w3": w3,
        }
        for i in range(NCORES)
    ]
    from concourse import bass_utils

    res = bass_utils.run_bass_kernel_spmd(nc, in_maps, core_ids=list(range(NCORES)))
    if _collect_results is not None:
        _collect_results.append(res)
    return np.concatenate([r["out"] for r in res.results], axis=0)
